# revision 31
# baseline (speedup 1.0000x reference)
"""Multi-head causal attention kernel for 8 Trainium2 NeuronCores.

Problem: B=128, T=256, C=384, H=6, D=64 (nn_MultiHeadAttention, causal).
Sharding: pure data-parallel over batch (16 batch elements per core, no
collectives); weights replicated.

Design (vs the 256us v1 baseline; measures ~133-137us on 8 cores):
minimize PE work, keep every other engine strictly below it, and
software-pipeline pairs so the PE never idles (HAM stays at K=8/8).

  * HOST-side prep: x pre-cast to bf16 and packed pair-major
    [B/2, C, 2T] (removes all 96 PE transposes + on-chip casts, halves
    input DMA bytes, and gives 1KB DMA lines); weights pre-cast/merged
    to single [128, 3*W] tiles (one wide DMA each); output returned as
    bf16 pair-major [B/2, C, 2T] and unpacked/upcast on the host.
  * batches processed in PAIRS (moving operands N=512).
  * v_aug per head = [ones64 | V_h] (128 cols): PV output rows 0:64
    hold the softmax row-sums replicated 64x, rows 64:128 the data, so
    normalization is ONE DVE reciprocal [64,512] + ONE DVE multiply
    [64,512] per head - no gpsimd partition_broadcast, no [1,512] row
    copies. (ones first: reciprocal_approx_fast silently misreads PSUM
    at base partition 64 on HW; tensor_mul handles any base fine.)
  * scores per (bi, head): layout [diagA | rect | diagB] in one PSUM
    bank; 2 matmuls (N=256 sharing the tk0 stationary + N=128). Heads
    are processed in pairs with K=64 row-packing (head A in array rows
    0:63, head B in 64:127) so both heads' score matmuls run
    concurrently and each LDWEIGHTS hides behind the other head's MM.
  * causal mask: exp() runs unmasked, then ONE gpsimd affine_select
    over a stride-2 block view zeros both 128x128 diagonal triangles
    of the bf16 P tile (exact zeros, no PE mask matmuls).
  * output projection flipped: y[c,t] = sum_hd wp[hd,c] * ot[hd,t], so
    the bias is per-partition -> folded into the ScalarE evacuation
    (activation Identity with bias AP).
  * software pipeline: pair pr+1's QT/KT/V projection groups are
    emitted between pair pr's attention sub-stages ((th, bi)
    granularity) so big N=512 matmuls fill every chain-latency bubble;
    every pair's y projection is emitted
    one stage late (inside the NEXT pair's attention) so its ot
    dependencies are already satisfied, and the last pair feeds its y
    matmuls incrementally per k-chunk to shorten the pipeline drain.

bf16 compute, fp32 accumulation in PSUM; PSUM split 3/3/2 banks
(projection+y / scores / PV).
"""

import sys

for p in ("/opt/trn_rl_repo",):
    if p not in sys.path:
        sys.path.insert(0, p)

import numpy as np
import ml_dtypes

import concourse.bass as bass
import concourse.mybir as mybir
import concourse.tile as tile
from concourse import bacc
from concourse.bass_utils import run_bass_kernel_spmd

P = 128
N_CORES = 8
B, T, C = 128, 256, 384
H, D = 6, 64
HD = H * D
B_LOC = B // N_CORES  # 16
KC = C // P           # 3 chunks over channels / head-pairs
T2 = 2 * T            # 512: pair width
SCALE = 1.0 / np.sqrt(D)

FP32 = mybir.dt.float32
BF16 = mybir.dt.bfloat16

BF16_NP = ml_dtypes.bfloat16


def build_kernel(nc: bass.Bass):
    NPAIR = B_LOC // 2
    # x HOST-prepped to pair-major [NPAIR, C, 2T] bf16 (1KB DMA lines)
    x = nc.dram_tensor("x", [NPAIR, C, T2], BF16, kind="ExternalInput").ap()
    # weights HOST-merged to [128, KC*HD] bf16 (chunk k at cols k*HD)
    wq = nc.dram_tensor("wq", [P, KC * HD], BF16, kind="ExternalInput").ap()
    wk = nc.dram_tensor("wk", [P, KC * HD], BF16, kind="ExternalInput").ap()
    wv = nc.dram_tensor("wv", [P, KC * HD], BF16, kind="ExternalInput").ap()
    wp = nc.dram_tensor("wp", [P, KC * C], BF16, kind="ExternalInput").ap()
    # bias HOST-pre-reshaped to [128, KC] fp32 (column m = chunk m)
    bpc = nc.dram_tensor("bpc", [P, KC], FP32, kind="ExternalInput").ap()
    # output pair-major [NPAIR, C, 2T] bf16; host unpacks to [B_LOC, T, C]
    out = nc.dram_tensor("out", [NPAIR, C, T2], BF16, kind="ExternalOutput").ap()

    with tile.TileContext(nc) as tc:
        from contextlib import ExitStack

        with ExitStack() as ctx:
            cpool = ctx.enter_context(tc.tile_pool(name="const", bufs=1))
            ps_big = ctx.enter_context(
                tc.tile_pool(name="psb", bufs=3, space="PSUM"))
            ps_s = ctx.enter_context(
                tc.tile_pool(name="pss", bufs=3, space="PSUM"))
            ps_pv = ctx.enter_context(
                tc.tile_pool(name="pspv", bufs=2, space="PSUM"))

            # ---- constants ----
            bp_sb = cpool.tile([P, KC], FP32, tag="bp_sb")
            nc.sync.dma_start(bp_sb[:], bpc[:, :])

            # ---- weights: ONE wide DMA per tensor (2.3KB lines);
            #      wq first so the first QT matmul group starts ASAP ----
            wq_sb, wk_sb, wv_sb, wp_sb = [], [], [], []
            for (dst, src, nm, w_) in ((wq_sb, wq, "wq", HD),
                                       (wk_sb, wk, "wk", HD),
                                       (wv_sb, wv, "wv", HD),
                                       (wp_sb, wp, "wp", C)):
                t_ = cpool.tile([P, KC * w_], BF16, tag=f"{nm}_all")
                nc.sync.dma_start(t_[:], src[:, :])
                for k in range(KC):
                    dst.append(t_[:, k * w_:(k + 1) * w_])

            # ---- persistent v_aug tiles (2 pair-slots x 2 bi x 2 i);
            #      ones half-columns written ONCE here ----
            v_aug = {}
            for sl in range(2):
                for bi in range(2):
                    for i in range(2):
                        t_ = cpool.tile([P, 2 * HD], BF16,
                                        tag=f"vaug{sl}{bi}{i}")
                        tv = t_[:].rearrange("p (h two d) -> p h two d",
                                             h=H, two=2)
                        # ones FIRST: row-sums land on PSUM partitions 0:64
                        # (reciprocal_approx_fast only works at base 0)
                        nc.vector.memset(tv[:, :, 0, :], 1.0)
                        v_aug[(sl, bi, i)] = t_

            # ---- pools ----
            xtpool = ctx.enter_context(tc.tile_pool(name="xt", bufs=9))
            qkpool = ctx.enter_context(tc.tile_pool(name="qk", bufs=12))
            ptpool = ctx.enter_context(tc.tile_pool(name="pt", bufs=12))
            otpool = ctx.enter_context(tc.tile_pool(name="ot", bufs=6))
            ypool = ctx.enter_context(tc.tile_pool(name="y", bufs=6))
            rbpool = ctx.enter_context(tc.tile_pool(name="rb", bufs=4))

            NP = B_LOC // 2

            def stage_xt(pr):
                xt = []
                for k in range(KC):
                    t_ = xtpool.tile([P, T2], BF16, tag="xt",
                                     name=f"xt{pr}_{k}")
                    nc.sync.dma_start(t_[:], x[pr, k * P:(k + 1) * P, :])
                    xt.append(t_)
                return xt

            def stage_proj_items(pr, xt):
                """10 closures: 6 QT/KT groups + 4 V groups (matmuls+evac)."""
                qt, kt = [None] * KC, [None] * KC
                items = []
                for wi, (dst, w_sb, nm) in enumerate(
                        ((qt, wq_sb, "qt"), (kt, wk_sb, "kt"))):
                    for m in range(KC):
                        def go(wi=wi, dst=dst, w_sb=w_sb, nm=nm, m=m):
                            ps = ps_big.tile([P, T2], FP32, tag="big",
                                             name=f"ps_{nm}{pr}_{m}")
                            for k in range(KC):
                                nc.tensor.matmul(
                                    ps[:], w_sb[k][:, m * P:(m + 1) * P],
                                    xt[k][:],
                                    start=(k == 0), stop=(k == KC - 1),
                                )
                            t_ = qkpool.tile([P, T2], BF16, tag="qk",
                                             name=f"{nm}{pr}_{m}")
                            if (wi * KC + m) % 2 == 0:
                                nc.scalar.copy(t_[:], ps[:])
                            else:
                                nc.vector.tensor_copy(t_[:], ps[:])
                            dst[m] = t_
                        items.append(go)
                sl = pr % 2
                for bi in range(2):
                    for i in range(2):
                        def gov(bi=bi, i=i):
                            ps = ps_big.tile([P, HD], FP32, tag="big",
                                             name=f"ps_v{pr}_{bi}{i}")
                            for k in range(KC):
                                nc.tensor.matmul(
                                    ps[:],
                                    xt[k][:, bi * T + i * P:
                                          bi * T + (i + 1) * P],
                                    wv_sb[k][:, :],
                                    start=(k == 0), stop=(k == KC - 1),
                                )
                            tv = v_aug[(sl, bi, i)][:].rearrange(
                                "p (h two d) -> p h two d", h=H, two=2)
                            src = ps[:].rearrange("p (h d) -> p h d", h=H)
                            if bi == 0:
                                nc.vector.tensor_copy(tv[:, :, 1, :], src)
                            else:
                                nc.scalar.copy(tv[:, :, 1, :], src)
                        items.append(gov)
                return items, qt, kt

            def stage_attn_th(pr, th, qt, kt, ot, between=None):
                sl = pr % 2
                pvt = {}
                for hh in range(2):
                    pvt[hh] = ps_pv.tile([P, T2], FP32, tag="pv",
                                         name=f"ps_pv{pr}_{th}{hh}")
                for bi in range(2):
                    if between is not None and bi == 1:
                        between()
                    pts = {}
                    pss = {}
                    # scores layout per unit: cols 0:128 diagA (tq0 x tk0),
                    # 128:256 diagB (tq1 x tk1), 256:384 rect (tq1 x tk0)
                    # -- one PSUM accumulation group; both diagonal blocks
                    # adjacent -> ONE affine_select.  The two heads' MMs
                    # are interleaved (different PE row groups) so each
                    # LDWEIGHTS hides behind the other head's matmul.
                    for hh in range(2):
                        pss[hh] = ps_s.tile([P, T + P], FP32, tag="s",
                                            name=f"ps_s{pr}_{th}{hh}")
                    # layout now [diagA | rect | diagB]: kh0 serves ONE
                    # N=256 matmul (diagA+rect adjacent); heads alternate
                    # so each LDWEIGHTS hides behind the other head's MM
                    for hh in range(2):
                        rows = slice(hh * 64, (hh + 1) * 64)
                        qh = qt[th][rows, bi * T:(bi + 1) * T]
                        kh = kt[th][rows, bi * T:(bi + 1) * T]
                        nc.tensor.matmul(
                            pss[hh][:, 0:T], kh[:, 0:P], qh[:, :],
                            start=True, stop=False)
                    for hh in range(2):
                        rows = slice(hh * 64, (hh + 1) * 64)
                        qh = qt[th][rows, bi * T:(bi + 1) * T]
                        kh = kt[th][rows, bi * T:(bi + 1) * T]
                        nc.tensor.matmul(
                            pss[hh][:, T:T + P], kh[:, P:T], qh[:, P:T],
                            start=False, stop=True)
                    for hh in range(2):
                        pt = ptpool.tile([P, T + P], BF16, tag="pt",
                                         name=f"pt{pr}_{th}{hh}")
                        nc.scalar.activation(
                            pt[:], pss[hh][:],
                            mybir.ActivationFunctionType.Exp,
                            scale=float(SCALE),
                        )
                        # zero the illegal triangles (tq < tk) of both
                        # diagonal blocks (cols 0:128 and 256:384) in one
                        # op via a stride-2 block view: keep col >= part
                        ptv = pt[:].rearrange("p (a c) -> p a c", c=P)[:, 0::2, :]
                        nc.gpsimd.affine_select(
                            out=ptv, in_=ptv,
                            compare_op=mybir.AluOpType.is_ge,
                            fill=0.0, base=0,
                            pattern=[[0, 2], [1, P]], channel_multiplier=-1,
                        )
                        pts[hh] = pt
                    for hh in range(2):
                        h = 2 * th + hh
                        # v0 consumes [diagA | rect] (pt cols 0:256) in one
                        # N=256 matmul -> pv cols [biT : biT+256]
                        nc.tensor.matmul(
                            pvt[hh][:, bi * T:(bi + 1) * T],
                            v_aug[(sl, bi, 0)][:, h * P:(h + 1) * P],
                            pts[hh][:, 0:T],
                            start=(bi == 0), stop=False)
                    for hh in range(2):
                        h = 2 * th + hh
                        nc.tensor.matmul(
                            pvt[hh][:, bi * T + P:(bi + 1) * T],
                            v_aug[(sl, bi, 1)][:, h * P:(h + 1) * P],
                            pts[hh][:, T:T + P],
                            start=False, stop=(bi == 1))
                # normalize: rows 0:64 of pvt hold row-sums replicated
                # 64x, rows 64:128 hold the unnormalized output
                for hh in range(2):
                    rb = rbpool.tile([64, T2], FP32, tag="rb",
                                     name=f"rb{pr}_{th}{hh}")
                    nc.vector.reciprocal_approx_fast(
                        rb[:], pvt[hh][0:64, :])
                    nc.vector.tensor_mul(
                        ot[th][hh * 64:(hh + 1) * 64, :],
                        pvt[hh][64:P, :], rb[:],
                    )

            def stage_y(pr, ot):
                for m in range(KC):
                    ps = ps_big.tile([P, T2], FP32, tag="big",
                                     name=f"ps_y{pr}_{m}")
                    for k in range(KC):
                        nc.tensor.matmul(
                            ps[:], wp_sb[k][:, m * P:(m + 1) * P], ot[k][:],
                            start=(k == 0), stop=(k == KC - 1),
                        )
                    y_sb = ypool.tile([P, T2], BF16, tag="y",
                                      name=f"y{pr}_{m}")
                    nc.scalar.activation(
                        y_sb[:], ps[:],
                        mybir.ActivationFunctionType.Identity,
                        bias=bp_sb[:, m:m + 1], scale=1.0,
                    )
                    nc.sync.dma_start(out[pr, m * P:(m + 1) * P, :],
                                      y_sb[:])

            # software pipeline: pair pr's attention interleaved with
            # pair pr+1's projection matmuls so the PE never idles
            xt0 = stage_xt(0)
            items, qt, kt = stage_proj_items(0, xt0)
            for it in items:
                it()
            pend_y = None
            for pr in range(NP):
                if pr + 1 < NP:
                    xt_n = stage_xt(pr + 1)
                    items_n, qt_n, kt_n = stage_proj_items(pr + 1, xt_n)
                else:
                    items_n, qt_n, kt_n = [], None, None
                ot = [otpool.tile([P, T2], BF16, tag="ot",
                                  name=f"ot{pr}_{k}") for k in range(KC)]
                # interleave next-pair projection groups at (th, bi)
                # granularity: one or two big matmul groups between every
                # attention sub-stage keeps the PE weight-load pipeline fed
                sched = [items_n[0:2], items_n[2:3], items_n[3:5],
                         items_n[5:6], items_n[6:8], items_n[8:10]]
                si = iter(sched)

                def emit_next():
                    for it in next(si, []):
                        it()
                if pr + 1 < NP:
                    for th in range(KC):
                        stage_attn_th(pr, th, qt, kt, ot, between=emit_next)
                        if th == 0 and pend_y is not None:
                            # previous pair's y projection lands here, when
                            # its ot dependencies are long satisfied
                            stage_y(*pend_y)
                            pend_y = None
                        emit_next()
                    pend_y = (pr, ot)
                else:
                    # last pair: no next-pair work to interleave; emit the
                    # y matmuls incrementally (k-chunk th as soon as ot[th]
                    # is ready) so the PE has work during the chain drain
                    ps_ys = [ps_big.tile([P, T2], FP32, tag="big",
                                         name=f"ps_yl{m}") for m in range(KC)]

                    def emit_y_chunk(th):
                        for m in range(KC):
                            nc.tensor.matmul(
                                ps_ys[m][:],
                                wp_sb[th][:, m * P:(m + 1) * P], ot[th][:],
                                start=(th == 0), stop=(th == KC - 1),
                            )
                    # emit y chunk th-1 during th's attention so its ot
                    # dependency is already satisfied (only th=2 tails)
                    for th in range(KC):
                        stage_attn_th(pr, th, qt, kt, ot)
                        if th == 0 and pend_y is not None:
                            stage_y(*pend_y)
                            pend_y = None
                        if th >= 1:
                            emit_y_chunk(th - 1)
                    emit_y_chunk(KC - 1)
                    for m in range(KC):
                        y_sb = ypool.tile([P, T2], BF16, tag="y",
                                          name=f"yl{m}")
                        nc.scalar.activation(
                            y_sb[:], ps_ys[m][:],
                            mybir.ActivationFunctionType.Identity,
                            bias=bp_sb[:, m:m + 1], scale=1.0,
                        )
                        nc.sync.dma_start(out[pr, m * P:(m + 1) * P, :],
                                          y_sb[:])
                qt, kt = qt_n, kt_n

    return nc


_CACHED = None


def _get_nc():
    global _CACHED
    if _CACHED is None:
        nc = bacc.Bacc("TRN2", target_bir_lowering=False, debug=False,
                       num_devices=N_CORES)
        build_kernel(nc)
        nc.compile()
        _CACHED = nc
    return _CACHED


def _ensure_ntff_hook():
    """This image's antenv lacks axon_hooks; shim it so trace=True works."""
    import types

    if "antenv.axon_hooks" in sys.modules:
        return
    mod = types.ModuleType("antenv.axon_hooks")
    _hook = [None]
    mod.set_axon_ntff_profile_hook = lambda h: _hook.__setitem__(0, h)
    mod.get_axon_ntff_profile_hook = lambda: _hook[0]
    sys.modules["antenv.axon_hooks"] = mod
    try:
        from trn_agent_boot.trn_boot import _ntff_profile_via_ctypes
        _hook[0] = _ntff_profile_via_ctypes("/opt/axon/libaxon_pjrt.so")
    except Exception:
        pass


def _w_merge(W):
    """[C, W] -> [128, KC*W]: chunk k (rows k*128:(k+1)*128) at cols k*W."""
    Wf = np.asarray(W, dtype=np.float32)
    w_ = Wf.shape[1]
    return np.ascontiguousarray(
        Wf.reshape(KC, P, w_).transpose(1, 0, 2).reshape(P, KC * w_)
    ).astype(BF16_NP)


def _prep_inputs(x, Wq, Wk, Wv, Wp, bp):
    """Host-side marshaling: transpose/cast/reshape the full inputs."""
    # [B, T, C] -> pair-major [B//2, C, 2T]
    xf = np.asarray(x, dtype=np.float32)
    Bn = xf.shape[0]
    xT = np.ascontiguousarray(
        xf.reshape(Bn // 2, 2, T, C).transpose(0, 3, 1, 2).reshape(
            Bn // 2, C, T2)).astype(BF16_NP)
    wq = _w_merge(np.asarray(Wq, dtype=np.float32)
                  .transpose(1, 0, 2).reshape(C, HD))
    wk = _w_merge(np.asarray(Wk, dtype=np.float32)
                  .transpose(1, 0, 2).reshape(C, HD))
    wv = _w_merge(np.asarray(Wv, dtype=np.float32)
                  .transpose(1, 0, 2).reshape(C, HD))
    wpc = _w_merge(np.asarray(Wp, dtype=np.float32))
    bpc = np.ascontiguousarray(
        np.asarray(bp, dtype=np.float32).reshape(KC, P).T)
    return xT, wq, wk, wv, wpc, bpc


def kernel(x, Wq, Wk, Wv, Wp, bp, _trace=False):
    if _trace:
        _ensure_ntff_hook()
    xT, wq, wk, wv, wpc, bpc = _prep_inputs(x, Wq, Wk, Wv, Wp, bp)
    nc = _get_nc()
    in_maps = []
    for c in range(N_CORES):
        npr = B_LOC // 2
        in_maps.append({
            "x": xT[c * npr:(c + 1) * npr],
            "wq": wq, "wk": wk, "wv": wv, "wp": wpc, "bpc": bpc,
        })
    res = run_bass_kernel_spmd(nc, in_maps, list(range(N_CORES)),
                               trace=_trace)
    y = np.concatenate(
        [np.asarray(res.results[c]["out"]) for c in range(N_CORES)], axis=0)
    # pair-major [B//2, C, 2T] bf16 -> [B, T, C] f32
    y = y.astype(np.float32).reshape(B // 2, C, 2, T).transpose(
        0, 2, 3, 1).reshape(B, T, C)
    y = np.ascontiguousarray(y)
    if _trace:
        return y, res
    return y


# revision 32
# speedup vs baseline: 1.0208x; 1.0208x over previous
"""Multi-head causal attention kernel for 8 Trainium2 NeuronCores.

Problem: B=128, T=256, C=384, H=6, D=64 (nn_MultiHeadAttention, causal).
Sharding: pure data-parallel over batch (16 batch elements per core, no
collectives); weights replicated.

Design (vs the 256us v1 baseline; measures ~133-137us on 8 cores):
minimize PE work, keep every other engine strictly below it, and
software-pipeline pairs so the PE never idles (HAM stays at K=8/8).

  * HOST-side prep: x pre-cast to bf16 and packed pair-major
    [B/2, C, 2T] (removes all 96 PE transposes + on-chip casts, halves
    input DMA bytes, and gives 1KB DMA lines); weights pre-cast/merged
    to single [128, 3*W] tiles (one wide DMA each); output returned as
    bf16 pair-major [B/2, C, 2T] and unpacked/upcast on the host.
  * batches processed in PAIRS (moving operands N=512).
  * v_aug per head = [ones64 | V_h] (128 cols): PV output rows 0:64
    hold the softmax row-sums replicated 64x, rows 64:128 the data, so
    normalization is ONE DVE reciprocal [64,512] + ONE DVE multiply
    [64,512] per head - no gpsimd partition_broadcast, no [1,512] row
    copies. (ones first: reciprocal_approx_fast silently misreads PSUM
    at base partition 64 on HW; tensor_mul handles any base fine.)
  * scores per (bi, head): layout [diagA | rect | diagB] in one PSUM
    bank; 2 matmuls (N=256 sharing the tk0 stationary + N=128). Heads
    are processed in pairs with K=64 row-packing (head A in array rows
    0:63, head B in 64:127) so both heads' score matmuls run
    concurrently and each LDWEIGHTS hides behind the other head's MM.
  * causal mask: exp() runs unmasked, then ONE gpsimd affine_select
    over a stride-2 block view zeros both 128x128 diagonal triangles
    of the bf16 P tile (exact zeros, no PE mask matmuls).
  * output projection flipped: y[c,t] = sum_hd wp[hd,c] * ot[hd,t], so
    the bias is per-partition -> folded into the ScalarE evacuation
    (activation Identity with bias AP).
  * software pipeline: pair pr+1's QT/KT/V projection groups are
    emitted between pair pr's attention sub-stages ((th, bi)
    granularity) so big N=512 matmuls fill every chain-latency bubble;
    ~70 junk warmup matmuls run during the initial DMA wait so HAM
    un-throttles before real work; every pair's y projection is emitted
    one stage late (inside the NEXT pair's attention) so its ot
    dependencies are already satisfied, and the last pair feeds its y
    matmuls incrementally per k-chunk to shorten the pipeline drain.

bf16 compute, fp32 accumulation in PSUM; PSUM split 3/3/2 banks
(projection+y / scores / PV).
"""

import sys

for p in ("/opt/trn_rl_repo",):
    if p not in sys.path:
        sys.path.insert(0, p)

import numpy as np
import ml_dtypes

import concourse.bass as bass
import concourse.mybir as mybir
import concourse.tile as tile
from concourse import bacc
from concourse.bass_utils import run_bass_kernel_spmd

P = 128
N_CORES = 8
B, T, C = 128, 256, 384
H, D = 6, 64
HD = H * D
B_LOC = B // N_CORES  # 16
KC = C // P           # 3 chunks over channels / head-pairs
T2 = 2 * T            # 512: pair width
SCALE = 1.0 / np.sqrt(D)

FP32 = mybir.dt.float32
BF16 = mybir.dt.bfloat16

BF16_NP = ml_dtypes.bfloat16


def build_kernel(nc: bass.Bass):
    NPAIR = B_LOC // 2
    # x HOST-prepped to pair-major [NPAIR, C, 2T] bf16 (1KB DMA lines)
    x = nc.dram_tensor("x", [NPAIR, C, T2], BF16, kind="ExternalInput").ap()
    # weights HOST-merged to [128, KC*HD] bf16 (chunk k at cols k*HD)
    wq = nc.dram_tensor("wq", [P, KC * HD], BF16, kind="ExternalInput").ap()
    wk = nc.dram_tensor("wk", [P, KC * HD], BF16, kind="ExternalInput").ap()
    wv = nc.dram_tensor("wv", [P, KC * HD], BF16, kind="ExternalInput").ap()
    wp = nc.dram_tensor("wp", [P, KC * C], BF16, kind="ExternalInput").ap()
    # bias HOST-pre-reshaped to [128, KC] fp32 (column m = chunk m)
    bpc = nc.dram_tensor("bpc", [P, KC], FP32, kind="ExternalInput").ap()
    # output pair-major [NPAIR, C, 2T] bf16; host unpacks to [B_LOC, T, C]
    out = nc.dram_tensor("out", [NPAIR, C, T2], BF16, kind="ExternalOutput").ap()

    with tile.TileContext(nc) as tc:
        from contextlib import ExitStack

        with ExitStack() as ctx:
            cpool = ctx.enter_context(tc.tile_pool(name="const", bufs=1))
            ps_big = ctx.enter_context(
                tc.tile_pool(name="psb", bufs=3, space="PSUM"))
            ps_s = ctx.enter_context(
                tc.tile_pool(name="pss", bufs=3, space="PSUM"))
            ps_pv = ctx.enter_context(
                tc.tile_pool(name="pspv", bufs=2, space="PSUM"))

            # ---- constants ----
            warm = cpool.tile([P, P], BF16, tag="warm")
            nc.vector.memset(warm[:], 0.0)
            bp_sb = cpool.tile([P, KC], FP32, tag="bp_sb")
            nc.sync.dma_start(bp_sb[:], bpc[:, :])

            # ---- weights: ONE wide DMA per tensor (2.3KB lines);
            #      wq first so the first QT matmul group starts ASAP ----
            wq_sb, wk_sb, wv_sb, wp_sb = [], [], [], []
            for (dst, src, nm, w_) in ((wq_sb, wq, "wq", HD),
                                       (wk_sb, wk, "wk", HD),
                                       (wv_sb, wv, "wv", HD),
                                       (wp_sb, wp, "wp", C)):
                t_ = cpool.tile([P, KC * w_], BF16, tag=f"{nm}_all")
                nc.sync.dma_start(t_[:], src[:, :])
                for k in range(KC):
                    dst.append(t_[:, k * w_:(k + 1) * w_])

            # ---- persistent v_aug tiles (2 pair-slots x 2 bi x 2 i);
            #      ones half-columns written ONCE here ----
            v_aug = {}
            for sl in range(2):
                for bi in range(2):
                    for i in range(2):
                        t_ = cpool.tile([P, 2 * HD], BF16,
                                        tag=f"vaug{sl}{bi}{i}")
                        tv = t_[:].rearrange("p (h two d) -> p h two d",
                                             h=H, two=2)
                        # ones FIRST: row-sums land on PSUM partitions 0:64
                        # (reciprocal_approx_fast only works at base 0)
                        nc.vector.memset(tv[:, :, 0, :], 1.0)
                        v_aug[(sl, bi, i)] = t_

            # ---- pools ----
            xtpool = ctx.enter_context(tc.tile_pool(name="xt", bufs=9))
            qkpool = ctx.enter_context(tc.tile_pool(name="qk", bufs=12))
            ptpool = ctx.enter_context(tc.tile_pool(name="pt", bufs=12))
            otpool = ctx.enter_context(tc.tile_pool(name="ot", bufs=6))
            ypool = ctx.enter_context(tc.tile_pool(name="y", bufs=6))
            rbpool = ctx.enter_context(tc.tile_pool(name="rb", bufs=4))

            NP = B_LOC // 2

            # ---- PE warmup: ~5us of junk matmuls during the initial
            #      input DMA wait, so HAM un-throttles (K=8/8) before the
            #      first real matmul ----
            ps_w = ps_big.tile([P, P], FP32, tag="big", name="ps_warm")
            NW = 70
            for i in range(NW):
                nc.tensor.matmul(ps_w[:], warm[:], warm[:],
                                 start=(i == 0), stop=(i == NW - 1))

            def stage_xt(pr):
                xt = []
                for k in range(KC):
                    t_ = xtpool.tile([P, T2], BF16, tag="xt",
                                     name=f"xt{pr}_{k}")
                    nc.sync.dma_start(t_[:], x[pr, k * P:(k + 1) * P, :])
                    xt.append(t_)
                return xt

            def stage_proj_items(pr, xt):
                """10 closures: 6 QT/KT groups + 4 V groups (matmuls+evac)."""
                qt, kt = [None] * KC, [None] * KC
                items = []
                for wi, (dst, w_sb, nm) in enumerate(
                        ((qt, wq_sb, "qt"), (kt, wk_sb, "kt"))):
                    for m in range(KC):
                        def go(wi=wi, dst=dst, w_sb=w_sb, nm=nm, m=m):
                            ps = ps_big.tile([P, T2], FP32, tag="big",
                                             name=f"ps_{nm}{pr}_{m}")
                            for k in range(KC):
                                nc.tensor.matmul(
                                    ps[:], w_sb[k][:, m * P:(m + 1) * P],
                                    xt[k][:],
                                    start=(k == 0), stop=(k == KC - 1),
                                )
                            t_ = qkpool.tile([P, T2], BF16, tag="qk",
                                             name=f"{nm}{pr}_{m}")
                            if (wi * KC + m) % 2 == 0:
                                nc.scalar.copy(t_[:], ps[:])
                            else:
                                nc.vector.tensor_copy(t_[:], ps[:])
                            dst[m] = t_
                        items.append(go)
                sl = pr % 2
                for bi in range(2):
                    for i in range(2):
                        def gov(bi=bi, i=i):
                            ps = ps_big.tile([P, HD], FP32, tag="big",
                                             name=f"ps_v{pr}_{bi}{i}")
                            for k in range(KC):
                                nc.tensor.matmul(
                                    ps[:],
                                    xt[k][:, bi * T + i * P:
                                          bi * T + (i + 1) * P],
                                    wv_sb[k][:, :],
                                    start=(k == 0), stop=(k == KC - 1),
                                )
                            tv = v_aug[(sl, bi, i)][:].rearrange(
                                "p (h two d) -> p h two d", h=H, two=2)
                            src = ps[:].rearrange("p (h d) -> p h d", h=H)
                            if bi == 0:
                                nc.vector.tensor_copy(tv[:, :, 1, :], src)
                            else:
                                nc.scalar.copy(tv[:, :, 1, :], src)
                        items.append(gov)
                return items, qt, kt

            def stage_attn_th(pr, th, qt, kt, ot, between=None):
                sl = pr % 2
                pvt = {}
                for hh in range(2):
                    pvt[hh] = ps_pv.tile([P, T2], FP32, tag="pv",
                                         name=f"ps_pv{pr}_{th}{hh}")
                for bi in range(2):
                    if between is not None and bi == 1:
                        between()
                    pts = {}
                    pss = {}
                    # scores layout per unit: cols 0:128 diagA (tq0 x tk0),
                    # 128:256 diagB (tq1 x tk1), 256:384 rect (tq1 x tk0)
                    # -- one PSUM accumulation group; both diagonal blocks
                    # adjacent -> ONE affine_select.  The two heads' MMs
                    # are interleaved (different PE row groups) so each
                    # LDWEIGHTS hides behind the other head's matmul.
                    for hh in range(2):
                        pss[hh] = ps_s.tile([P, T + P], FP32, tag="s",
                                            name=f"ps_s{pr}_{th}{hh}")
                    # layout now [diagA | rect | diagB]: kh0 serves ONE
                    # N=256 matmul (diagA+rect adjacent); heads alternate
                    # so each LDWEIGHTS hides behind the other head's MM
                    for hh in range(2):
                        rows = slice(hh * 64, (hh + 1) * 64)
                        qh = qt[th][rows, bi * T:(bi + 1) * T]
                        kh = kt[th][rows, bi * T:(bi + 1) * T]
                        nc.tensor.matmul(
                            pss[hh][:, 0:T], kh[:, 0:P], qh[:, :],
                            start=True, stop=False)
                    for hh in range(2):
                        rows = slice(hh * 64, (hh + 1) * 64)
                        qh = qt[th][rows, bi * T:(bi + 1) * T]
                        kh = kt[th][rows, bi * T:(bi + 1) * T]
                        nc.tensor.matmul(
                            pss[hh][:, T:T + P], kh[:, P:T], qh[:, P:T],
                            start=False, stop=True)
                    for hh in range(2):
                        pt = ptpool.tile([P, T + P], BF16, tag="pt",
                                         name=f"pt{pr}_{th}{hh}")
                        nc.scalar.activation(
                            pt[:], pss[hh][:],
                            mybir.ActivationFunctionType.Exp,
                            scale=float(SCALE),
                        )
                        # zero the illegal triangles (tq < tk) of both
                        # diagonal blocks (cols 0:128 and 256:384) in one
                        # op via a stride-2 block view: keep col >= part
                        ptv = pt[:].rearrange("p (a c) -> p a c", c=P)[:, 0::2, :]
                        nc.gpsimd.affine_select(
                            out=ptv, in_=ptv,
                            compare_op=mybir.AluOpType.is_ge,
                            fill=0.0, base=0,
                            pattern=[[0, 2], [1, P]], channel_multiplier=-1,
                        )
                        pts[hh] = pt
                    for hh in range(2):
                        h = 2 * th + hh
                        # v0 consumes [diagA | rect] (pt cols 0:256) in one
                        # N=256 matmul -> pv cols [biT : biT+256]
                        nc.tensor.matmul(
                            pvt[hh][:, bi * T:(bi + 1) * T],
                            v_aug[(sl, bi, 0)][:, h * P:(h + 1) * P],
                            pts[hh][:, 0:T],
                            start=(bi == 0), stop=False)
                    for hh in range(2):
                        h = 2 * th + hh
                        nc.tensor.matmul(
                            pvt[hh][:, bi * T + P:(bi + 1) * T],
                            v_aug[(sl, bi, 1)][:, h * P:(h + 1) * P],
                            pts[hh][:, T:T + P],
                            start=False, stop=(bi == 1))
                # normalize: rows 0:64 of pvt hold row-sums replicated
                # 64x, rows 64:128 hold the unnormalized output
                for hh in range(2):
                    rb = rbpool.tile([64, T2], FP32, tag="rb",
                                     name=f"rb{pr}_{th}{hh}")
                    nc.vector.reciprocal_approx_fast(
                        rb[:], pvt[hh][0:64, :])
                    nc.vector.tensor_mul(
                        ot[th][hh * 64:(hh + 1) * 64, :],
                        pvt[hh][64:P, :], rb[:],
                    )

            def stage_y(pr, ot):
                for m in range(KC):
                    ps = ps_big.tile([P, T2], FP32, tag="big",
                                     name=f"ps_y{pr}_{m}")
                    for k in range(KC):
                        nc.tensor.matmul(
                            ps[:], wp_sb[k][:, m * P:(m + 1) * P], ot[k][:],
                            start=(k == 0), stop=(k == KC - 1),
                        )
                    y_sb = ypool.tile([P, T2], BF16, tag="y",
                                      name=f"y{pr}_{m}")
                    nc.scalar.activation(
                        y_sb[:], ps[:],
                        mybir.ActivationFunctionType.Identity,
                        bias=bp_sb[:, m:m + 1], scale=1.0,
                    )
                    nc.sync.dma_start(out[pr, m * P:(m + 1) * P, :],
                                      y_sb[:])

            # software pipeline: pair pr's attention interleaved with
            # pair pr+1's projection matmuls so the PE never idles
            xt0 = stage_xt(0)
            items, qt, kt = stage_proj_items(0, xt0)
            for it in items:
                it()
            pend_y = None
            for pr in range(NP):
                if pr + 1 < NP:
                    xt_n = stage_xt(pr + 1)
                    items_n, qt_n, kt_n = stage_proj_items(pr + 1, xt_n)
                else:
                    items_n, qt_n, kt_n = [], None, None
                ot = [otpool.tile([P, T2], BF16, tag="ot",
                                  name=f"ot{pr}_{k}") for k in range(KC)]
                # interleave next-pair projection groups at (th, bi)
                # granularity: one or two big matmul groups between every
                # attention sub-stage keeps the PE weight-load pipeline fed
                sched = [items_n[0:2], items_n[2:3], items_n[3:5],
                         items_n[5:6], items_n[6:8], items_n[8:10]]
                si = iter(sched)

                def emit_next():
                    for it in next(si, []):
                        it()
                if pr + 1 < NP:
                    for th in range(KC):
                        stage_attn_th(pr, th, qt, kt, ot, between=emit_next)
                        if th == 0 and pend_y is not None:
                            # previous pair's y projection lands here, when
                            # its ot dependencies are long satisfied
                            stage_y(*pend_y)
                            pend_y = None
                        emit_next()
                    pend_y = (pr, ot)
                else:
                    # last pair: no next-pair work to interleave; emit the
                    # y matmuls incrementally (k-chunk th as soon as ot[th]
                    # is ready) so the PE has work during the chain drain
                    ps_ys = [ps_big.tile([P, T2], FP32, tag="big",
                                         name=f"ps_yl{m}") for m in range(KC)]

                    def emit_y_chunk(th):
                        for m in range(KC):
                            nc.tensor.matmul(
                                ps_ys[m][:],
                                wp_sb[th][:, m * P:(m + 1) * P], ot[th][:],
                                start=(th == 0), stop=(th == KC - 1),
                            )
                    # emit y chunk th-1 during th's attention so its ot
                    # dependency is already satisfied (only th=2 tails)
                    for th in range(KC):
                        stage_attn_th(pr, th, qt, kt, ot)
                        if th == 0 and pend_y is not None:
                            stage_y(*pend_y)
                            pend_y = None
                        if th >= 1:
                            emit_y_chunk(th - 1)
                    emit_y_chunk(KC - 1)
                    for m in range(KC):
                        y_sb = ypool.tile([P, T2], BF16, tag="y",
                                          name=f"yl{m}")
                        nc.scalar.activation(
                            y_sb[:], ps_ys[m][:],
                            mybir.ActivationFunctionType.Identity,
                            bias=bp_sb[:, m:m + 1], scale=1.0,
                        )
                        nc.sync.dma_start(out[pr, m * P:(m + 1) * P, :],
                                          y_sb[:])
                qt, kt = qt_n, kt_n

    return nc


_CACHED = None


def _get_nc():
    global _CACHED
    if _CACHED is None:
        nc = bacc.Bacc("TRN2", target_bir_lowering=False, debug=False,
                       num_devices=N_CORES)
        build_kernel(nc)
        nc.compile()
        _CACHED = nc
    return _CACHED


def _ensure_ntff_hook():
    """This image's antenv lacks axon_hooks; shim it so trace=True works."""
    import types

    if "antenv.axon_hooks" in sys.modules:
        return
    mod = types.ModuleType("antenv.axon_hooks")
    _hook = [None]
    mod.set_axon_ntff_profile_hook = lambda h: _hook.__setitem__(0, h)
    mod.get_axon_ntff_profile_hook = lambda: _hook[0]
    sys.modules["antenv.axon_hooks"] = mod
    try:
        from trn_agent_boot.trn_boot import _ntff_profile_via_ctypes
        _hook[0] = _ntff_profile_via_ctypes("/opt/axon/libaxon_pjrt.so")
    except Exception:
        pass


def _w_merge(W):
    """[C, W] -> [128, KC*W]: chunk k (rows k*128:(k+1)*128) at cols k*W."""
    Wf = np.asarray(W, dtype=np.float32)
    w_ = Wf.shape[1]
    return np.ascontiguousarray(
        Wf.reshape(KC, P, w_).transpose(1, 0, 2).reshape(P, KC * w_)
    ).astype(BF16_NP)


def _prep_inputs(x, Wq, Wk, Wv, Wp, bp):
    """Host-side marshaling: transpose/cast/reshape the full inputs."""
    # [B, T, C] -> pair-major [B//2, C, 2T]
    xf = np.asarray(x, dtype=np.float32)
    Bn = xf.shape[0]
    xT = np.ascontiguousarray(
        xf.reshape(Bn // 2, 2, T, C).transpose(0, 3, 1, 2).reshape(
            Bn // 2, C, T2)).astype(BF16_NP)
    wq = _w_merge(np.asarray(Wq, dtype=np.float32)
                  .transpose(1, 0, 2).reshape(C, HD))
    wk = _w_merge(np.asarray(Wk, dtype=np.float32)
                  .transpose(1, 0, 2).reshape(C, HD))
    wv = _w_merge(np.asarray(Wv, dtype=np.float32)
                  .transpose(1, 0, 2).reshape(C, HD))
    wpc = _w_merge(np.asarray(Wp, dtype=np.float32))
    bpc = np.ascontiguousarray(
        np.asarray(bp, dtype=np.float32).reshape(KC, P).T)
    return xT, wq, wk, wv, wpc, bpc


def kernel(x, Wq, Wk, Wv, Wp, bp, _trace=False):
    if _trace:
        _ensure_ntff_hook()
    xT, wq, wk, wv, wpc, bpc = _prep_inputs(x, Wq, Wk, Wv, Wp, bp)
    nc = _get_nc()
    in_maps = []
    for c in range(N_CORES):
        npr = B_LOC // 2
        in_maps.append({
            "x": xT[c * npr:(c + 1) * npr],
            "wq": wq, "wk": wk, "wv": wv, "wp": wpc, "bpc": bpc,
        })
    res = run_bass_kernel_spmd(nc, in_maps, list(range(N_CORES)),
                               trace=_trace)
    y = np.concatenate(
        [np.asarray(res.results[c]["out"]) for c in range(N_CORES)], axis=0)
    # pair-major [B//2, C, 2T] bf16 -> [B, T, C] f32
    y = y.astype(np.float32).reshape(B // 2, C, 2, T).transpose(
        0, 2, 3, 1).reshape(B, T, C)
    y = np.ascontiguousarray(y)
    if _trace:
        return y, res
    return y
